# revision 4
# baseline (speedup 1.0000x reference)
"""AWGN channel kernel for Trainium2: y = x + sqrt(1/SNR) * noise.

Full inputs x, noise: (16384, 4096) float32. Row-sharded across 8
NeuronCores (data parallel, 2048 rows/core); each core streams 16
chunks of [128, 4096] through SBUF and computes the fused
(noise * STD) + x in one DVE scalar_tensor_tensor op per chunk.
"""

import numpy as np

N_CORES = 8
ROWS, COLS = 16384, 4096
SHARD_ROWS = ROWS // N_CORES  # 2048
P = 128
N_CHUNKS = SHARD_ROWS // P  # 16
SNR = 10.0
STD = float(np.sqrt(1.0 / SNR))

import os

CHUNK_COLS = int(os.environ.get("K_CHUNK_COLS", "4096"))
BUFS = int(os.environ.get("K_BUFS", "3"))
LOAD_ENGINES = os.environ.get("K_LOAD_ENGINES", "sync,sync")  # x,noise
STORE_ENGINE = os.environ.get("K_STORE_ENGINE", "scalar")

_cache = {}


def _build():
    if "nc" in _cache:
        return _cache["nc"]

    import concourse.tile as tile
    from concourse import bacc, mybir

    nc = bacc.Bacc(
        "TRN2",
        target_bir_lowering=False,
        debug=False,
        num_devices=N_CORES,
    )
    x_ap = nc.dram_tensor(
        "x", [SHARD_ROWS, COLS], mybir.dt.float32, kind="ExternalInput"
    ).ap()
    n_ap = nc.dram_tensor(
        "noise", [SHARD_ROWS, COLS], mybir.dt.float32, kind="ExternalInput"
    ).ap()
    y_ap = nc.dram_tensor(
        "y", [SHARD_ROWS, COLS], mybir.dt.float32, kind="ExternalOutput"
    ).ap()

    eng_x, eng_n = (getattr(nc, e) for e in LOAD_ENGINES.split(","))
    eng_y = getattr(nc, STORE_ENGINE)

    # view DRAM as flat contiguous chunks of [P, CHUNK_COLS]
    def _view(ap):
        if CHUNK_COLS == COLS:
            return ap.rearrange("(c p) f -> c p f", p=P)
        if CHUNK_COLS > COLS:
            r = CHUNK_COLS // COLS
            return ap.rearrange("(c p r) f -> c p (r f)", r=r, p=P)
        s = COLS // CHUNK_COLS
        return ap.rearrange("(c q) (s f) -> c (q s) f", s=s, q=P // s)

    total = SHARD_ROWS * COLS
    n_chunks = total // (P * CHUNK_COLS)
    x_v = _view(x_ap)
    n_v = _view(n_ap)
    y_v = _view(y_ap)

    with tile.TileContext(nc) as tc:
        with (
            tc.tile_pool(name="xp", bufs=BUFS) as xp,
            tc.tile_pool(name="npool", bufs=BUFS) as npool,
            tc.tile_pool(name="yp", bufs=BUFS) as yp,
        ):
            for c in range(n_chunks):
                xt = xp.tile([P, CHUNK_COLS], mybir.dt.float32)
                nt = npool.tile([P, CHUNK_COLS], mybir.dt.float32)
                yt = yp.tile([P, CHUNK_COLS], mybir.dt.float32)
                eng_x.dma_start(out=xt[:], in_=x_v[c])
                eng_n.dma_start(out=nt[:], in_=n_v[c])
                nc.vector.scalar_tensor_tensor(
                    out=yt[:],
                    in0=nt[:],
                    scalar=STD,
                    in1=xt[:],
                    op0=mybir.AluOpType.mult,
                    op1=mybir.AluOpType.add,
                )
                eng_y.dma_start(out=y_v[c], in_=yt[:])

    nc.compile()
    _cache["nc"] = nc
    return nc


def _run(x, noise, trace=False, tmpdir=None):
    from concourse.bass_utils import run_bass_kernel_spmd

    nc = _build()
    x = np.ascontiguousarray(x, dtype=np.float32)
    noise = np.ascontiguousarray(noise, dtype=np.float32)
    in_maps = [
        {
            "x": x[i * SHARD_ROWS : (i + 1) * SHARD_ROWS],
            "noise": noise[i * SHARD_ROWS : (i + 1) * SHARD_ROWS],
        }
        for i in range(N_CORES)
    ]
    res = run_bass_kernel_spmd(
        nc, in_maps, list(range(N_CORES)), trace=trace, tmpdir=tmpdir
    )
    out = np.concatenate([res.results[i]["y"] for i in range(N_CORES)], axis=0)
    return out, res


def kernel(x, noise):
    out, _ = _run(x, noise)
    return out
